# revision 1
# baseline (speedup 1.0000x reference)
"""Trainium2 Bass kernel for batched 3-D k-NN local-covariance trace.

Problem: pcd [B=8, N=4096, 3] -> per-point trace of the 3x3 covariance of its
k=5 nearest neighbors (self included), normalized by the per-batch max.

Sharding: data-parallel over batch — core b owns batch b (N=4096 points).

Per-core algorithm (all SBUF-resident after the initial load):
  * rank value r[i,j] = 2*x_i.x_j - |x_i|^2 - |x_j|^2 = -d2[i,j], computed as a
    single K=5 augmented matmul:  lhsT rows [2x,2y,2z,-sq,1], rhs rows
    [x,y,z,1,-sq].  Row-block of 128 queries x 8 chunks of 512 candidates.
  * top-5 neighbors per query via DVE max (top-8 values) + max_index
    (first-occurrence indices, ties resolve to the lowest index like
    jax.lax.top_k).
  * neighbor coordinate gather via gpsimd indirect_copy: the idxs tile is read
    wrapped per 16-partition core group in (slot-major, query-minor) order, so
    passing the max_index tile [:, :5] directly makes each group gather its own
    16 queries' neighbors from a table with coords on partitions 16g..16g+2.
  * stable centered trace per query (sum of squared deviations from the
    5-neighbor mean), components summed across partitions with a tiny matmul
    against a constant selection matrix E.
  * global max over the 4096 traces (gpsimd partition_all_reduce) -> scale by
    1/(max+1e-8) -> DMA out.
"""

import numpy as np
from contextlib import ExitStack

N = 4096
KNN = 5
P = 128          # queries per row block
NBLK = N // P    # 32 row blocks
CH = 512         # candidate chunk (one fp32 PSUM bank)
NCH = N // CH    # 8 chunks
G16 = 16         # partitions per gpsimd core group
NG = P // G16    # 8 groups per row block


def build_nc():
    import concourse.bass as bass
    import concourse.tile as tile
    from concourse import bacc, mybir
    from concourse import bass_isa

    dt = mybir.dt
    f32 = dt.float32
    Alu = mybir.AluOpType
    Axis = mybir.AxisListType

    nc = bacc.Bacc("TRN2", target_bir_lowering=False, debug=False)
    pcd_d = nc.dram_tensor("pcd", [N, 3], f32, kind="ExternalInput")
    out_d = nc.dram_tensor("out", [N], f32, kind="ExternalOutput")
    pcd_t = pcd_d.ap().rearrange("n d -> d n")      # [3, N] strided view

    with tile.TileContext(nc) as tc, ExitStack() as ctx:
        const = ctx.enter_context(tc.tile_pool(name="const", bufs=1))
        mpool = ctx.enter_context(tc.tile_pool(name="mval", bufs=2))
        small = ctx.enter_context(tc.tile_pool(name="small", bufs=3))
        psum = ctx.enter_context(tc.tile_pool(name="psum", bufs=6, space="PSUM"))
        psacc = ctx.enter_context(tc.tile_pool(name="psacc", bufs=1, space="PSUM"))

        # ---- one-time setup -------------------------------------------------
        xr = const.tile([5, N], f32)         # rhs rows [x,y,z,1,-sq]
        xl = const.tile([5, N], f32)         # lhsT rows [2x,2y,2z,-sq,1]
        tbl = const.tile([P, N], f32)        # gather table: coords on p%16<3

        # coords into xr/xl rows 0-2 straight from DRAM (parallel queues);
        # per-row DMAs so transfers overlap instead of queueing on one engine
        for d, eng in enumerate((nc.sync, nc.scalar, nc.gpsimd)):
            eng.dma_start(xr[d:d + 1, :], pcd_t[d:d + 1, :])
        for d, eng in enumerate((nc.scalar, nc.gpsimd, nc.sync)):
            eng.dma_start(xl[d:d + 1, :], pcd_t[d:d + 1, :])
        nc.gpsimd.memset(tbl[:], 0.0)
        nc.scalar.dma_start(tbl[0:3, :], pcd_t)

        nc.scalar.mul(xl[0:3, :], xl[0:3, :], 2.0)

        s3 = const.tile([3, N], f32)         # squared coords
        nc.vector.tensor_mul(s3[:], xr[0:3, :], xr[0:3, :])

        ones3 = const.tile([3, 1], f32)
        nc.vector.memset(ones3[:], 1.0)
        ones1 = const.tile([1, N], f32)
        nc.vector.memset(ones1[:], 1.0)

        sq_neg = const.tile([1, N], f32)
        for c in range(NCH):
            sl = slice(c * CH, (c + 1) * CH)
            sq_ps = psum.tile([1, CH], f32, tag="mm")
            nc.tensor.matmul(sq_ps[:], ones3[:], s3[:, sl], start=True, stop=True)
            nc.scalar.mul(sq_neg[0:1, sl], sq_ps[:], -1.0)

        # assemble remaining rows via DMA (arbitrary partition offsets),
        # spread across engine queues so they run concurrently
        nc.sync.dma_start(xr[3:4, :], ones1[:])
        nc.gpsimd.dma_start(xr[4:5, :], sq_neg[:])
        nc.scalar.dma_start(xl[3:4, :], sq_neg[:])
        nc.sync.dma_start(xl[4:5, :], ones1[:])

        # replicate coords to every 16-partition group of tbl
        engs = (nc.sync, nc.scalar, nc.gpsimd, nc.sync,
                nc.scalar, nc.gpsimd, nc.sync)
        for g in range(1, NG):
            engs[g - 1].dma_start(tbl[G16 * g:G16 * g + 3, :], tbl[0:3, :])

        # E[p, g] = 1 iff p//16 == g and p%16 < 3  (component-sum selector)
        esel = const.tile([P, NG], f32)
        nc.vector.memset(esel[:], 0.0)
        for g in range(NG):
            nc.sync.dma_start(esel[G16 * g:G16 * g + 3, g:g + 1], ones3[:])

        trace_ps = psacc.tile([G16, NG * NBLK], f32)   # [16, 256], one bank

        # ---- main loop over row blocks -------------------------------------
        for r in range(NBLK):
            lhsT = xl[:, r * P:(r + 1) * P]
            mval = mpool.tile([P, N], f32)
            for c in range(NCH):
                sl = slice(c * CH, (c + 1) * CH)
                ps = psum.tile([P, CH], f32, tag="mm")
                nc.tensor.matmul(ps[:], lhsT, xr[:, sl], start=True, stop=True)
                nc.scalar.copy(mval[:, sl], ps[:])

            v8 = small.tile([P, 8], f32, tag="v8")
            nc.vector.max(v8[:], mval[:])
            idx8 = small.tile([P, 8], dt.uint16, tag="idx8")
            nc.vector.max_index(idx8[:], v8[:], mval[:])

            # gather: group g gathers, for its 16 queries, slot-major:
            # gath[p, s*16+q16] = tbl[p, idx8[16*(p//16)+q16, s]]
            gath = small.tile([P, KNN * G16], f32, tag="gath")
            nc.gpsimd.indirect_copy(gath[:], tbl[:], idx8[:, :KNN], True)

            gv = gath[:].rearrange("p (s q) -> p q s", s=KNN, q=G16)
            ssum = small.tile([P, G16], f32, tag="ssum")
            nc.vector.tensor_reduce(ssum[:], gv, axis=Axis.X, op=Alu.add)
            mean = small.tile([P, G16], f32, tag="mean")
            nc.scalar.mul(mean[:], ssum[:], 1.0 / KNN)

            cent = small.tile([P, G16, KNN], f32, tag="cent")
            nc.gpsimd.tensor_sub(cent[:], gv,
                                 mean[:].unsqueeze(2).broadcast_to([P, G16, KNN]))
            nc.gpsimd.tensor_mul(cent[:], cent[:], cent[:])
            tt = small.tile([P, G16], f32, tag="tt")
            nc.vector.tensor_reduce(tt[:], cent[:], axis=Axis.X, op=Alu.add)

            nc.tensor.matmul(trace_ps[:, r * NG:(r + 1) * NG], tt[:], esel[:],
                             start=True, stop=True)

        # ---- normalize + store ---------------------------------------------
        tr_sb = const.tile([G16, NG * NBLK], f32)
        nc.scalar.copy(tr_sb[:], trace_ps[:])
        gmax = const.tile([G16, 1], f32)
        nc.vector.tensor_reduce(gmax[:], tr_sb[:], axis=Axis.X, op=Alu.max)
        gmax_all = const.tile([G16, 1], f32)
        nc.gpsimd.partition_all_reduce(gmax_all[:], gmax[:], channels=G16,
                                       reduce_op=bass_isa.ReduceOp.max)
        denom = const.tile([G16, 1], f32)
        nc.vector.tensor_scalar_add(denom[:], gmax_all[:], 1e-8)
        rec = const.tile([G16, 1], f32)
        nc.vector.reciprocal(rec[:], denom[:])
        outv = const.tile([G16, NG * NBLK], f32)
        nc.vector.tensor_scalar_mul(outv[:], tr_sb[:], rec[:])

        nc.sync.dma_start(
            out_d.ap().rearrange("(r g q) -> q (r g)", r=NBLK, g=NG, q=G16),
            outv[:],
        )

    nc.compile()
    return nc


_NC_CACHE = {}


def kernel(pcd, k):
    pcd = np.asarray(pcd)
    k = int(np.asarray(k))
    assert k == KNN, f"kernel hardcodes k={KNN}, got {k}"
    B, n, d = pcd.shape
    assert (n, d) == (N, 3), f"kernel hardcodes N={N}, got {(n, d)}"

    from concourse.bass_utils import run_bass_kernel_spmd

    if "nc" not in _NC_CACHE:
        _NC_CACHE["nc"] = build_nc()
    nc = _NC_CACHE["nc"]

    in_maps = [{"pcd": np.ascontiguousarray(pcd[b], dtype=np.float32)}
               for b in range(B)]
    res = run_bass_kernel_spmd(nc, in_maps, list(range(B)))
    out = np.stack([res.results[b]["out"] for b in range(B)], axis=0)
    return out.astype(np.float32, copy=False)


if __name__ == "__main__":
    x = np.random.randn(8, N, 3).astype(np.float32)
    y = kernel(x, 5)
    print(y.shape, y.dtype, y[:2, :4])



# revision 12
# speedup vs baseline: 1.6575x; 1.6575x over previous
"""Trainium2 Bass kernel for batched 3-D k-NN local-covariance trace.

Problem: pcd [B=8, N=4096, 3] -> per-point trace of the 3x3 covariance of its
k=5 nearest neighbors (self included), normalized by the per-batch max.

Sharding: data-parallel over batch — core b owns batch b (N=4096 points).

Per-core algorithm (all SBUF-resident after the initial load):
  * rank value r[i,j] = 2*x_i.x_j - |x_i|^2 - |x_j|^2 = -d2[i,j], computed as
    an fp16 hi/lo-split augmented matmul (K=13) that streams 1 col/cycle on
    the PE (4x faster than fp32) while keeping ~fp32 accuracy: x = h + l with
    h = fp16(x), l = fp16(x - h); 2x_i*x_j = 2h_i*h_j + 2l_i*h_j + 2h_i*l_j
    (the dropped 2*l_i*l_j term is ~1e-6); |x|^2 split the same way.
  * top-5 neighbors per query via DVE max (top-8 values) + max_index.
  * neighbor gather via gpsimd indirect_copy from a table holding coords on
    partitions 16g+{0,1,2} and |x|^2 on 16g+3 — the sq row rides the same
    gather for free.
  * trace via S1/S2: trace = S_sq - |S1|^2/5, assembled by two tiny
    PSUM-accumulated selection matmuls per row block.
  * global max (gpsimd partition_all_reduce) -> scale -> DMA out.

Point order: the kernel works in transposed order n~ = (n%32)*128 + n//32
(n = original point index) because the setup pipeline stages pcd as
[128 partitions, 32 points x 3] and PE-transposes it; the selection/gather
are order-agnostic and the output DMA pattern maps back to original order.
"""

import numpy as np
from contextlib import ExitStack

N = 4096
KNN = 5
P = 128          # queries per row block
NBLK = N // P    # 32 row blocks
CH = 512         # candidate chunk (one fp32 PSUM bank)
NCH = N // CH    # 8 chunks
G16 = 16         # partitions per gpsimd core group
NG = P // G16    # 8 groups per row block
CPP = 32         # points staged per partition (N / 128)


def build_nc():
    import concourse.bass as bass
    import concourse.tile as tile
    from concourse import bacc, mybir
    from concourse import bass_isa

    dt = mybir.dt
    f32 = dt.float32
    f16 = dt.float16
    Alu = mybir.AluOpType
    Axis = mybir.AxisListType

    nc = bacc.Bacc("TRN2", target_bir_lowering=False, debug=False)
    pcd_d = nc.dram_tensor("pcd", [N, 3], f32, kind="ExternalInput")
    out_d = nc.dram_tensor("out", [N], f32, kind="ExternalOutput")

    with tile.TileContext(nc) as tc, ExitStack() as ctx:
        const = ctx.enter_context(tc.tile_pool(name="const", bufs=1))
        setup = ctx.enter_context(tc.tile_pool(name="setup", bufs=1))
        mpool = ctx.enter_context(tc.tile_pool(name="mval", bufs=2))
        small = ctx.enter_context(tc.tile_pool(name="small", bufs=3))
        psum = ctx.enter_context(tc.tile_pool(name="psum", bufs=6, space="PSUM"))
        pstr = ctx.enter_context(tc.tile_pool(name="pstr", bufs=1, space="PSUM"))
        psacc = ctx.enter_context(tc.tile_pool(name="psacc", bufs=1, space="PSUM"))

        # ---- stage pcd: one contiguous DMA, [p, c*3+d] = pcd[32p+c, d] ------
        stage = setup.tile([P, 3 * CPP], f32)
        nc.sync.dma_start(
            stage[:], pcd_d.ap().rearrange("(p c) d -> p (c d)", p=P, c=CPP))

        # identities for PE transposes (diagonal via affine_select)
        ident16 = const.tile([P, P], f16)
        nc.vector.memset(ident16[:], 1.0)
        nc.gpsimd.affine_select(ident16[:], ident16[:], [[1, P]],
                                Alu.is_equal, 0.0, base=0, channel_multiplier=-1)
        ident32 = const.tile([P, P], f32)
        nc.vector.memset(ident32[:], 1.0)
        nc.gpsimd.affine_select(ident32[:], ident32[:], [[1, P]],
                                Alu.is_equal, 0.0, base=0, channel_multiplier=-1)

        # ---- fp16 hi/lo split of coords (tiny [128, 96] elementwise ops) ----
        h16 = setup.tile([P, 3 * CPP], f16)
        nc.vector.tensor_copy(h16[:], stage[:])                  # h = fp16(x)
        hf = setup.tile([P, 3 * CPP], f32)
        nc.vector.tensor_copy(hf[:], h16[:])
        lf = setup.tile([P, 3 * CPP], f32)
        nc.vector.tensor_sub(lf[:], stage[:], hf[:])             # l = x - h
        l16 = setup.tile([P, 3 * CPP], f16)
        nc.vector.tensor_copy(l16[:], lf[:])

        # ---- |x|^2 per point, then hi/lo split of -sq -----------------------
        sqc = setup.tile([P, 3 * CPP], f32)
        nc.scalar.square(sqc[:], stage[:])
        sqp = setup.tile([P, CPP], f32)                          # +|x|^2
        nc.vector.tensor_reduce(sqp[:], sqc[:].rearrange("p (c d) -> p c d", d=3),
                                axis=Axis.X, op=Alu.add)
        msq = setup.tile([P, CPP], f32)
        nc.scalar.mul(msq[:], sqp[:], -1.0)
        pack2 = setup.tile([P, 2 * CPP], f16)                    # [msh | msl]
        nc.vector.tensor_copy(pack2[:, 0:CPP], msq[:])           # msh = fp16(-sq)
        mshf = setup.tile([P, CPP], f32)
        nc.vector.tensor_copy(mshf[:], pack2[:, 0:CPP])
        mslf = setup.tile([P, CPP], f32)
        nc.vector.tensor_sub(mslf[:], msq[:], mshf[:])
        nc.vector.tensor_copy(pack2[:, CPP:2 * CPP], mslf[:])    # msl

        # ---- PE transposes to candidate-row layout --------------------------
        # input free dims pre-permuted "(c d) -> (d c)" so transposed rows come
        # out coord-blocked: rows [32d + c] = coord d of point (c,p)
        def tr(src_view, pdim, dtype, ident):
            ps_t = pstr.tile([pdim, P], src_view.dtype, tag="tr")
            nc.tensor.transpose(ps_t[:], src_view, ident)
            sb = setup.tile([pdim, P], dtype)
            nc.scalar.copy(sb[:], ps_t[:])
            return sb

        # materialize the (d c) permutation first: the PE transpose input AP
        # must have a single free dimension on hardware
        hperm = setup.tile([P, 3 * CPP], f16)
        nc.vector.tensor_copy(hperm[:], h16[:].rearrange("p (c d) -> p d c", d=3))
        lperm = setup.tile([P, 3 * CPP], f16)
        nc.vector.tensor_copy(lperm[:], l16[:].rearrange("p (c d) -> p d c", d=3))
        fperm = setup.tile([P, 3 * CPP], f32)
        nc.gpsimd.tensor_copy(fperm[:], stage[:].rearrange("p (c d) -> p d c", d=3))

        Th = tr(hperm[:], 3 * CPP, f16, ident16[:])
        Tl = tr(lperm[:], 3 * CPP, f16, ident16[:])
        Tf = tr(fperm[:], 3 * CPP, f32, ident32[:])
        T2 = tr(pack2[:], 2 * CPP, f16, ident16[:])
        Tq = tr(sqp[:], CPP, f32, ident32[:])

        # ---- operand tiles [13, N] fp16 (rows via fast 256B-run DMAs) -------
        # matmul terms (contraction k):
        #   k 0-2 : 2h_i * h_j      k 3-5 : 2l_i * h_j     k 6-8 : 2h_i * l_j
        #   k 9-10: (-sq_i hi/lo)*1 k 11-12: 1*(-sq_j hi/lo)
        xrh = const.tile([13, N], f16)       # rhs  rows [h,h,l,1,msq]
        xlh = const.tile([13, N], f16)       # lhsT rows [2h,2l,2h,msq,1]
        # 2x-scaled transposed tiles (compute stays at partition offset 0;
        # odd-partition row placement goes through DMAs, which allow any
        # partition offset)
        Th2 = setup.tile([3 * CPP, P], f16)
        nc.vector.tensor_scalar_mul(Th2[:], Th[:], 2.0)          # exact in fp16
        Tl2 = setup.tile([3 * CPP, P], f16)
        nc.vector.tensor_scalar_mul(Tl2[:], Tl[:], 2.0)
        ones_row = const.tile([1, N], f16)
        nc.vector.memset(ones_row[:], 1.0)

        engs = (nc.sync, nc.scalar, nc.gpsimd)

        def row(dst_tile, r0, src):
            return dst_tile[r0:r0 + 1, :].rearrange("r (c p) -> r c p", c=CPP), src

        for d in range(3):
            engs[d].dma_start(*row(xrh, d, Th[32 * d:32 * (d + 1), :]))
            engs[d].dma_start(*row(xrh, 3 + d, Th[32 * d:32 * (d + 1), :]))
            engs[d].dma_start(*row(xrh, 6 + d, Tl[32 * d:32 * (d + 1), :]))
            engs[d].dma_start(*row(xlh, d, Th2[32 * d:32 * (d + 1), :]))
            engs[d].dma_start(*row(xlh, 3 + d, Tl2[32 * d:32 * (d + 1), :]))
            engs[d].dma_start(*row(xlh, 6 + d, Th2[32 * d:32 * (d + 1), :]))
        nc.sync.dma_start(xrh[9:10, :], ones_row[:])
        nc.scalar.dma_start(xrh[10:11, :], ones_row[:])
        nc.sync.dma_start(*row(xrh, 11, T2[0:CPP, :]))
        nc.scalar.dma_start(*row(xrh, 12, T2[CPP:2 * CPP, :]))
        nc.gpsimd.dma_start(*row(xlh, 9, T2[0:CPP, :]))
        nc.sync.dma_start(*row(xlh, 10, T2[CPP:2 * CPP, :]))
        nc.scalar.dma_start(xlh[11:12, :], ones_row[:])
        nc.gpsimd.dma_start(xlh[12:13, :], ones_row[:])

        # ---- gather table: coords on 16g+{0,1,2}, |x|^2 on 16g+3 ------------
        tbl = const.tile([P, N], f32)
        nc.gpsimd.memset(tbl[:], 0.0)
        for g in range(NG):
            eng = engs[g % 3]
            for d in range(3):
                eng.dma_start(
                    tbl[G16 * g + d:G16 * g + d + 1, :].rearrange("r (c p) -> r c p", c=CPP),
                    Tf[32 * d:32 * (d + 1), :])
            eng.dma_start(
                tbl[G16 * g + 3:G16 * g + 4, :].rearrange("r (c p) -> r c p", c=CPP),
                Tq[:])

        # ---- selection matmul rhs: E_sq (row 16g+3 -> +1), E_xyz (-1/5) -----
        esel_sq = const.tile([P, NG], f32)
        nc.vector.memset(esel_sq[:], 0.0)
        esel_xyz = const.tile([P, NG], f32)
        nc.vector.memset(esel_xyz[:], 0.0)
        one1 = const.tile([1, 1], f32)
        nc.vector.memset(one1[:], 1.0)
        mfifth = const.tile([3, 1], f32)
        nc.vector.memset(mfifth[:], -1.0 / KNN)
        for g in range(NG):
            engs[g % 3].dma_start(esel_sq[G16 * g + 3:G16 * g + 4, g:g + 1], one1[:])
            engs[g % 3].dma_start(esel_xyz[G16 * g:G16 * g + 3, g:g + 1], mfifth[:])

        trace_ps = psacc.tile([G16, NG * NBLK], f32)   # [16, 256], one bank

        # ---- main loop over row blocks -------------------------------------
        for r in range(NBLK):
            lhsT = xlh[:, r * P:(r + 1) * P]
            mval = mpool.tile([P, N], f32)
            for c in range(NCH):
                sl = slice(c * CH, (c + 1) * CH)
                ps = psum.tile([P, CH], f32, tag="mm")
                nc.tensor.matmul(ps[:], lhsT, xrh[:, sl], start=True, stop=True)
                nc.scalar.copy(mval[:, sl], ps[:])

            v8 = small.tile([P, 8], f32, tag="v8")
            nc.vector.max(v8[:], mval[:])
            idx8 = small.tile([P, 8], dt.uint16, tag="idx8")
            nc.vector.max_index(idx8[:], v8[:], mval[:])

            # gather: group g gathers, for its 16 queries, slot-major:
            # gath[p, s*16+q16] = tbl[p, idx8[16*(p//16)+q16, s]]
            gath = small.tile([P, KNN * G16], f32, tag="gath")
            nc.gpsimd.indirect_copy(gath[:], tbl[:], idx8[:, :KNN], True)

            gv = gath[:].rearrange("p (s q) -> p q s", s=KNN, q=G16)
            S = small.tile([P, G16], f32, tag="S")
            nc.vector.tensor_reduce(S[:], gv, axis=Axis.X, op=Alu.add)
            S2 = small.tile([P, G16], f32, tag="S2")
            nc.gpsimd.tensor_mul(S2[:], S[:], S[:])

            osl = slice(r * NG, (r + 1) * NG)
            nc.tensor.matmul(trace_ps[:, osl], S[:], esel_sq[:],
                             start=True, stop=False)
            nc.tensor.matmul(trace_ps[:, osl], S2[:], esel_xyz[:],
                             start=False, stop=True)

        # ---- normalize + store ---------------------------------------------
        tr_sb = const.tile([G16, NG * NBLK], f32)
        nc.scalar.copy(tr_sb[:], trace_ps[:])
        gmax = const.tile([G16, 1], f32)
        nc.vector.tensor_reduce(gmax[:], tr_sb[:], axis=Axis.X, op=Alu.max)
        gmax_all = const.tile([G16, 1], f32)
        nc.gpsimd.partition_all_reduce(gmax_all[:], gmax[:], channels=G16,
                                       reduce_op=bass_isa.ReduceOp.max)
        denom = const.tile([G16, 1], f32)
        nc.vector.tensor_scalar_add(denom[:], gmax_all[:], 1e-8)
        rec = const.tile([G16, 1], f32)
        nc.vector.reciprocal(rec[:], denom[:])
        outv = const.tile([G16, NG * NBLK], f32)
        nc.vector.tensor_scalar_mul(outv[:], tr_sb[:], rec[:])

        # query n~ = 128r + 16g + q maps to original n = 512g + 32q + r
        nc.sync.dma_start(
            out_d.ap().rearrange("(g q r) -> q r g", g=NG, q=G16, r=NBLK),
            outv[:].rearrange("q (r g) -> q r g", r=NBLK, g=NG),
        )

    nc.compile()
    return nc


_NC_CACHE = {}


def kernel(pcd, k):
    pcd = np.asarray(pcd)
    k = int(np.asarray(k))
    assert k == KNN, f"kernel hardcodes k={KNN}, got {k}"
    B, n, d = pcd.shape
    assert (n, d) == (N, 3), f"kernel hardcodes N={N}, got {(n, d)}"

    from concourse.bass_utils import run_bass_kernel_spmd

    if "nc" not in _NC_CACHE:
        _NC_CACHE["nc"] = build_nc()
    nc = _NC_CACHE["nc"]

    in_maps = [{"pcd": np.ascontiguousarray(pcd[b], dtype=np.float32)}
               for b in range(B)]
    res = run_bass_kernel_spmd(nc, in_maps, list(range(B)))
    out = np.stack([res.results[b]["out"] for b in range(B)], axis=0)
    return out.astype(np.float32, copy=False)


if __name__ == "__main__":
    x = np.random.randn(8, N, 3).astype(np.float32)
    y = kernel(x, 5)
    print(y.shape, y.dtype, y[:2, :4])


# revision 15
# speedup vs baseline: 1.9899x; 1.2006x over previous
"""Trainium2 Bass kernel for batched 3-D k-NN local-covariance trace.

Problem: pcd [B=8, N=4096, 3] -> per-point trace of the 3x3 covariance of its
k=5 nearest neighbors (self included), normalized by the per-batch max.

Sharding: data-parallel over batch — core b owns batch b (N=4096 points).

Per-core algorithm (all SBUF-resident after the initial load):
  * rank value r[i,j] = 2*x_i.x_j - |x_i|^2 - |x_j|^2 = -d2[i,j], computed as
    an fp16 hi/lo-split augmented matmul (K=13) that streams 1 col/cycle on
    the PE (4x faster than fp32) while keeping ~fp32 accuracy: x = h + l with
    h = fp16(x), l = fp16(x - h); 2x_i*x_j = 2h_i*h_j + 2l_i*h_j + 2h_i*l_j
    (the dropped 2*l_i*l_j term is ~1e-6); |x|^2 split the same way.
  * top-5 neighbors per query via DVE max (top-8 values) + max_index.
  * neighbor gather via gpsimd indirect_copy from a table holding coords on
    partitions 16g+{0,1,2} and |x|^2 on 16g+3 — the sq row rides the same
    gather for free.
  * trace via S1/S2: trace = S_sq - |S1|^2/5, assembled by two tiny
    PSUM-accumulated selection matmuls per row block.
  * global max (gpsimd partition_all_reduce) -> scale -> DMA out.

Point order: the kernel works in transposed order n~ = (n%32)*128 + n//32
(n = original point index) because the setup pipeline stages pcd as
[128 partitions, 32 points x 3] and PE-transposes it; the selection/gather
are order-agnostic and the output DMA pattern maps back to original order.
"""

import numpy as np
from contextlib import ExitStack

N = 4096
KNN = 5
P = 128          # queries per row block
NBLK = N // P    # 32 row blocks
CH = 512         # candidate chunk (one fp32 PSUM bank)
NCH = N // CH    # 8 chunks
G16 = 16         # partitions per gpsimd core group
NG = P // G16    # 8 groups per row block
CPP = 32         # points staged per partition (N / 128)


def build_nc():
    import concourse.bass as bass
    import concourse.tile as tile
    from concourse import bacc, mybir
    from concourse import bass_isa

    dt = mybir.dt
    f32 = dt.float32
    f16 = dt.float16
    Alu = mybir.AluOpType
    Axis = mybir.AxisListType

    nc = bacc.Bacc("TRN2", target_bir_lowering=False, debug=False)
    pcd_d = nc.dram_tensor("pcd", [N, 3], f32, kind="ExternalInput")
    out_d = nc.dram_tensor("out", [N], f32, kind="ExternalOutput")

    with tile.TileContext(nc) as tc, ExitStack() as ctx:
        const = ctx.enter_context(tc.tile_pool(name="const", bufs=1))
        setup = ctx.enter_context(tc.tile_pool(name="setup", bufs=1))
        mpool = ctx.enter_context(tc.tile_pool(name="mval", bufs=2))
        small = ctx.enter_context(tc.tile_pool(name="small", bufs=3))
        psum = ctx.enter_context(tc.tile_pool(name="psum", bufs=6, space="PSUM"))
        pstr = ctx.enter_context(tc.tile_pool(name="pstr", bufs=1, space="PSUM"))
        psacc = ctx.enter_context(tc.tile_pool(name="psacc", bufs=1, space="PSUM"))

        # ---- stage pcd: one contiguous DMA, [p, c*3+d] = pcd[32p+c, d] ------
        stage = setup.tile([P, 3 * CPP], f32)
        nc.sync.dma_start(
            stage[:], pcd_d.ap().rearrange("(p c) d -> p (c d)", p=P, c=CPP))

        # identities for PE transposes (diagonal via affine_select)
        ident16 = const.tile([P, P], f16)
        nc.vector.memset(ident16[:], 1.0)
        nc.gpsimd.affine_select(ident16[:], ident16[:], [[1, P]],
                                Alu.is_equal, 0.0, base=0, channel_multiplier=-1)
        ident32 = const.tile([P, P], f32)
        nc.vector.memset(ident32[:], 1.0)
        nc.gpsimd.affine_select(ident32[:], ident32[:], [[1, P]],
                                Alu.is_equal, 0.0, base=0, channel_multiplier=-1)

        # ---- fp16 hi/lo split of coords (tiny [128, 96] elementwise ops) ----
        h16 = setup.tile([P, 3 * CPP], f16)
        nc.vector.tensor_copy(h16[:], stage[:])                  # h = fp16(x)
        hf = setup.tile([P, 3 * CPP], f32)
        nc.vector.tensor_copy(hf[:], h16[:])
        lf = setup.tile([P, 3 * CPP], f32)
        nc.vector.tensor_sub(lf[:], stage[:], hf[:])             # l = x - h
        l16 = setup.tile([P, 3 * CPP], f16)
        nc.vector.tensor_copy(l16[:], lf[:])

        # ---- |x|^2 per point, then hi/lo split of -sq -----------------------
        sqc = setup.tile([P, 3 * CPP], f32)
        nc.scalar.square(sqc[:], stage[:])
        sqp = setup.tile([P, CPP], f32)                          # +|x|^2
        nc.vector.tensor_reduce(sqp[:], sqc[:].rearrange("p (c d) -> p c d", d=3),
                                axis=Axis.X, op=Alu.add)
        msq = setup.tile([P, CPP], f32)
        nc.scalar.mul(msq[:], sqp[:], -1.0)
        pack2 = setup.tile([P, 2 * CPP], f16)                    # [msh | msl]
        nc.vector.tensor_copy(pack2[:, 0:CPP], msq[:])           # msh = fp16(-sq)
        mshf = setup.tile([P, CPP], f32)
        nc.vector.tensor_copy(mshf[:], pack2[:, 0:CPP])
        mslf = setup.tile([P, CPP], f32)
        nc.vector.tensor_sub(mslf[:], msq[:], mshf[:])
        nc.vector.tensor_copy(pack2[:, CPP:2 * CPP], mslf[:])    # msl

        # ---- PE transposes to candidate-row layout --------------------------
        # input free dims pre-permuted "(c d) -> (d c)" so transposed rows come
        # out coord-blocked: rows [32d + c] = coord d of point (c,p)
        def tr(src_view, pdim, dtype, ident):
            ps_t = pstr.tile([pdim, P], src_view.dtype, tag="tr")
            nc.tensor.transpose(ps_t[:], src_view, ident)
            sb = setup.tile([pdim, P], dtype)
            nc.scalar.copy(sb[:], ps_t[:])
            return sb

        # materialize the (d c) permutation first: the PE transpose input AP
        # must have a single free dimension on hardware
        hperm = setup.tile([P, 3 * CPP], f16)
        nc.vector.tensor_copy(hperm[:], h16[:].rearrange("p (c d) -> p d c", d=3))
        lperm = setup.tile([P, 3 * CPP], f16)
        nc.vector.tensor_copy(lperm[:], l16[:].rearrange("p (c d) -> p d c", d=3))
        fperm = setup.tile([P, 3 * CPP], f32)
        nc.gpsimd.tensor_copy(fperm[:], stage[:].rearrange("p (c d) -> p d c", d=3))

        Th = tr(hperm[:], 3 * CPP, f16, ident16[:])
        Tl = tr(lperm[:], 3 * CPP, f16, ident16[:])
        Tf = tr(fperm[:], 3 * CPP, f32, ident32[:])
        T2 = tr(pack2[:], 2 * CPP, f16, ident16[:])
        Tq = tr(sqp[:], CPP, f32, ident32[:])

        # ---- operand tiles [13, N] fp16 (rows via fast 256B-run DMAs) -------
        # matmul terms (contraction k):
        #   k 0-2 : 2h_i * h_j      k 3-5 : 2l_i * h_j     k 6-8 : 2h_i * l_j
        #   k 9-10: (-sq_i hi/lo)*1 k 11-12: 1*(-sq_j hi/lo)
        xrh = const.tile([13, N], f16)       # rhs  rows [h,h,l,1,msq]
        xlh = const.tile([13, N], f16)       # lhsT rows [2h,2l,2h,msq,1]
        # 2x-scaled transposed tiles (compute stays at partition offset 0;
        # odd-partition row placement goes through DMAs, which allow any
        # partition offset)
        Th2 = setup.tile([3 * CPP, P], f16)
        nc.vector.tensor_scalar_mul(Th2[:], Th[:], 2.0)          # exact in fp16
        Tl2 = setup.tile([3 * CPP, P], f16)
        nc.vector.tensor_scalar_mul(Tl2[:], Tl[:], 2.0)
        ones_row = const.tile([1, N], f16)
        nc.vector.memset(ones_row[:], 1.0)

        engs = (nc.sync, nc.scalar, nc.gpsimd)

        def row(dst_tile, r0, src):
            return dst_tile[r0:r0 + 1, :].rearrange("r (c p) -> r c p", c=CPP), src

        for d in range(3):
            engs[d].dma_start(*row(xrh, d, Th[32 * d:32 * (d + 1), :]))
            engs[d].dma_start(*row(xrh, 3 + d, Th[32 * d:32 * (d + 1), :]))
            engs[d].dma_start(*row(xrh, 6 + d, Tl[32 * d:32 * (d + 1), :]))
            engs[d].dma_start(*row(xlh, d, Th2[32 * d:32 * (d + 1), :]))
            engs[d].dma_start(*row(xlh, 3 + d, Tl2[32 * d:32 * (d + 1), :]))
            engs[d].dma_start(*row(xlh, 6 + d, Th2[32 * d:32 * (d + 1), :]))
        nc.sync.dma_start(xrh[9:10, :], ones_row[:])
        nc.scalar.dma_start(xrh[10:11, :], ones_row[:])
        nc.sync.dma_start(*row(xrh, 11, T2[0:CPP, :]))
        nc.scalar.dma_start(*row(xrh, 12, T2[CPP:2 * CPP, :]))
        nc.gpsimd.dma_start(*row(xlh, 9, T2[0:CPP, :]))
        nc.sync.dma_start(*row(xlh, 10, T2[CPP:2 * CPP, :]))
        nc.scalar.dma_start(xlh[11:12, :], ones_row[:])
        nc.gpsimd.dma_start(xlh[12:13, :], ones_row[:])

        # ---- gather table: coords on 16g+{0,1,2}, |x|^2 on 16g+3 ------------
        tbl = const.tile([P, N], f32)
        nc.gpsimd.memset(tbl[:], 0.0)
        for g in range(NG):
            eng = engs[g % 3]
            for d in range(3):
                eng.dma_start(
                    tbl[G16 * g + d:G16 * g + d + 1, :].rearrange("r (c p) -> r c p", c=CPP),
                    Tf[32 * d:32 * (d + 1), :])
            eng.dma_start(
                tbl[G16 * g + 3:G16 * g + 4, :].rearrange("r (c p) -> r c p", c=CPP),
                Tq[:])

        # ---- selection matmul rhs: E_sq (row 16g+3 -> +1), E_xyz (-1/5) -----
        esel_sq = const.tile([P, NG], f32)
        nc.vector.memset(esel_sq[:], 0.0)
        esel_xyz = const.tile([P, NG], f32)
        nc.vector.memset(esel_xyz[:], 0.0)
        one1 = const.tile([1, 1], f32)
        nc.vector.memset(one1[:], 1.0)
        mfifth = const.tile([3, 1], f32)
        nc.vector.memset(mfifth[:], -1.0 / KNN)
        for g in range(NG):
            engs[g % 3].dma_start(esel_sq[G16 * g + 3:G16 * g + 4, g:g + 1], one1[:])
            engs[g % 3].dma_start(esel_xyz[G16 * g:G16 * g + 3, g:g + 1], mfifth[:])

        trace_ps = psacc.tile([G16, NG * NBLK], f32)   # [16, 256], one bank

        # ---- main loop over row blocks -------------------------------------
        for r in range(NBLK):
            lhsT = xlh[:, r * P:(r + 1) * P]
            mval = mpool.tile([P, N], f32)
            for c in range(NCH):
                sl = slice(c * CH, (c + 1) * CH)
                ps = psum.tile([P, CH], f32, tag="mm")
                nc.tensor.matmul(ps[:], lhsT, xrh[:, sl], start=True, stop=True)
                nc.scalar.copy(mval[:, sl], ps[:])

            v8 = small.tile([P, 8], f32, tag="v8")
            nc.vector.max(v8[:], mval[:])
            idx8 = small.tile([P, 8], dt.uint16, tag="idx8")
            nc.vector.max_index(idx8[:], v8[:], mval[:])

            # gather: group g gathers, for its 16 queries, slot-major:
            # gath[p, s*16+q16] = tbl[p, idx8[16*(p//16)+q16, s]]
            gath = small.tile([P, KNN * G16], f32, tag="gath")
            nc.gpsimd.indirect_copy(gath[:], tbl[:], idx8[:, :KNN], True)

            gv = gath[:].rearrange("p (s q) -> p q s", s=KNN, q=G16)
            S = small.tile([P, G16], f32, tag="S")
            nc.vector.tensor_reduce(S[:], gv, axis=Axis.X, op=Alu.add)
            S2 = small.tile([P, G16], f32, tag="S2")
            nc.gpsimd.tensor_mul(S2[:], S[:], S[:])

            osl = slice(r * NG, (r + 1) * NG)
            nc.tensor.matmul(trace_ps[:, osl], S[:], esel_sq[:],
                             start=True, stop=False)
            nc.tensor.matmul(trace_ps[:, osl], S2[:], esel_xyz[:],
                             start=False, stop=True)

        # ---- normalize + store ---------------------------------------------
        tr_sb = const.tile([G16, NG * NBLK], f32)
        nc.scalar.copy(tr_sb[:], trace_ps[:])
        gmax = const.tile([G16, 1], f32)
        nc.vector.tensor_reduce(gmax[:], tr_sb[:], axis=Axis.X, op=Alu.max)
        gmax_t_ps = pstr.tile([1, G16], f32, tag="tr")
        nc.tensor.transpose(gmax_t_ps[:], gmax[:], ident32[:G16, :G16])
        gmax_t = const.tile([1, G16], f32)
        nc.scalar.copy(gmax_t[:], gmax_t_ps[:])
        gmax1 = const.tile([1, 1], f32)
        nc.vector.tensor_reduce(gmax1[:], gmax_t[:], axis=Axis.X, op=Alu.max)
        denom1 = const.tile([1, 1], f32)
        nc.vector.tensor_scalar_add(denom1[:], gmax1[:], 1e-8)
        rec1 = const.tile([1, 1], f32)
        nc.vector.reciprocal(rec1[:], denom1[:])
        ones_col = const.tile([1, G16], f32)
        nc.vector.memset(ones_col[:], 1.0)
        rec_ps = pstr.tile([G16, 1], f32, tag="tr")
        nc.tensor.matmul(rec_ps[:], ones_col[:], rec1[:], start=True, stop=True)
        rec = const.tile([G16, 1], f32)
        nc.scalar.copy(rec[:], rec_ps[:])
        outv = const.tile([G16, NG * NBLK], f32)
        nc.vector.tensor_scalar_mul(outv[:], tr_sb[:], rec[:])

        # query n~ = 128r + 16g + q maps to original n = 512g + 32q + r
        nc.sync.dma_start(
            out_d.ap().rearrange("(g q r) -> q r g", g=NG, q=G16, r=NBLK),
            outv[:].rearrange("q (r g) -> q r g", r=NBLK, g=NG),
        )

    nc.compile()
    return nc


_NC_CACHE = {}


def kernel(pcd, k):
    pcd = np.asarray(pcd)
    k = int(np.asarray(k))
    assert k == KNN, f"kernel hardcodes k={KNN}, got {k}"
    B, n, d = pcd.shape
    assert (n, d) == (N, 3), f"kernel hardcodes N={N}, got {(n, d)}"

    from concourse.bass_utils import run_bass_kernel_spmd

    if "nc" not in _NC_CACHE:
        _NC_CACHE["nc"] = build_nc()
    nc = _NC_CACHE["nc"]

    in_maps = [{"pcd": np.ascontiguousarray(pcd[b], dtype=np.float32)}
               for b in range(B)]
    res = run_bass_kernel_spmd(nc, in_maps, list(range(B)))
    out = np.stack([res.results[b]["out"] for b in range(B)], axis=0)
    return out.astype(np.float32, copy=False)


if __name__ == "__main__":
    x = np.random.randn(8, N, 3).astype(np.float32)
    y = kernel(x, 5)
    print(y.shape, y.dtype, y[:2, :4])


# revision 16
# speedup vs baseline: 2.0274x; 1.0189x over previous
"""Trainium2 Bass kernel for batched 3-D k-NN local-covariance trace.

Problem: pcd [B=8, N=4096, 3] -> per-point trace of the 3x3 covariance of its
k=5 nearest neighbors (self included), normalized by the per-batch max.

Sharding: data-parallel over batch — core b owns batch b (N=4096 points).

Per-core algorithm (all SBUF-resident after the initial load):
  * rank value r[i,j] = 2*x_i.x_j - |x_i|^2 - |x_j|^2 = -d2[i,j], computed as
    an fp16 hi/lo-split augmented matmul (K=13) that streams 1 col/cycle on
    the PE (4x faster than fp32) while keeping ~fp32 accuracy: x = h + l with
    h = fp16(x), l = fp16(x - h); 2x_i*x_j = 2h_i*h_j + 2l_i*h_j + 2h_i*l_j
    (the dropped 2*l_i*l_j term is ~1e-6); |x|^2 split the same way.
  * top-5 neighbors per query via DVE max (top-8 values) + max_index.
  * neighbor gather via gpsimd indirect_copy from a table holding coords on
    partitions 16g+{0,1,2} and |x|^2 on 16g+3 — the sq row rides the same
    gather for free.
  * trace via S1/S2: trace = S_sq - |S1|^2/5, assembled by two tiny
    PSUM-accumulated selection matmuls per row block.
  * global max (gpsimd partition_all_reduce) -> scale -> DMA out.

Point order: the kernel works in transposed order n~ = (n%32)*128 + n//32
(n = original point index) because the setup pipeline stages pcd as
[128 partitions, 32 points x 3] and PE-transposes it; the selection/gather
are order-agnostic and the output DMA pattern maps back to original order.
"""

import numpy as np
from contextlib import ExitStack

N = 4096
KNN = 5
P = 128          # queries per row block
NBLK = N // P    # 32 row blocks
CH = 512         # candidate chunk (one fp32 PSUM bank)
NCH = N // CH    # 8 chunks
G16 = 16         # partitions per gpsimd core group
NG = P // G16    # 8 groups per row block
CPP = 32         # points staged per partition (N / 128)


def build_nc():
    import concourse.bass as bass
    import concourse.tile as tile
    from concourse import bacc, mybir
    from concourse import bass_isa

    dt = mybir.dt
    f32 = dt.float32
    f16 = dt.float16
    Alu = mybir.AluOpType
    Axis = mybir.AxisListType

    nc = bacc.Bacc("TRN2", target_bir_lowering=False, debug=False)
    pcd_d = nc.dram_tensor("pcd", [N, 3], f32, kind="ExternalInput")
    out_d = nc.dram_tensor("out", [N], f32, kind="ExternalOutput")

    with tile.TileContext(nc) as tc, ExitStack() as ctx:
        const = ctx.enter_context(tc.tile_pool(name="const", bufs=1))
        setup = ctx.enter_context(tc.tile_pool(name="setup", bufs=1))
        mpool = ctx.enter_context(tc.tile_pool(name="mval", bufs=3))
        small = ctx.enter_context(tc.tile_pool(name="small", bufs=3))
        psum = ctx.enter_context(tc.tile_pool(name="psum", bufs=6, space="PSUM"))
        pstr = ctx.enter_context(tc.tile_pool(name="pstr", bufs=1, space="PSUM"))
        psacc = ctx.enter_context(tc.tile_pool(name="psacc", bufs=1, space="PSUM"))

        # ---- stage pcd: one contiguous DMA, [p, c*3+d] = pcd[32p+c, d] ------
        stage = setup.tile([P, 3 * CPP], f32)
        nc.sync.dma_start(
            stage[:], pcd_d.ap().rearrange("(p c) d -> p (c d)", p=P, c=CPP))

        # identities for PE transposes (diagonal via affine_select)
        ident16 = const.tile([P, P], f16)
        nc.vector.memset(ident16[:], 1.0)
        nc.gpsimd.affine_select(ident16[:], ident16[:], [[1, P]],
                                Alu.is_equal, 0.0, base=0, channel_multiplier=-1)
        ident32 = const.tile([P, P], f32)
        nc.vector.memset(ident32[:], 1.0)
        nc.gpsimd.affine_select(ident32[:], ident32[:], [[1, P]],
                                Alu.is_equal, 0.0, base=0, channel_multiplier=-1)

        # ---- fp16 hi/lo split of coords (tiny [128, 96] elementwise ops) ----
        h16 = setup.tile([P, 3 * CPP], f16)
        nc.vector.tensor_copy(h16[:], stage[:])                  # h = fp16(x)
        hf = setup.tile([P, 3 * CPP], f32)
        nc.vector.tensor_copy(hf[:], h16[:])
        lf = setup.tile([P, 3 * CPP], f32)
        nc.vector.tensor_sub(lf[:], stage[:], hf[:])             # l = x - h
        l16 = setup.tile([P, 3 * CPP], f16)
        nc.vector.tensor_copy(l16[:], lf[:])

        # ---- |x|^2 per point, then hi/lo split of -sq -----------------------
        sqc = setup.tile([P, 3 * CPP], f32)
        nc.scalar.square(sqc[:], stage[:])
        sqp = setup.tile([P, CPP], f32)                          # +|x|^2
        nc.vector.tensor_reduce(sqp[:], sqc[:].rearrange("p (c d) -> p c d", d=3),
                                axis=Axis.X, op=Alu.add)
        msq = setup.tile([P, CPP], f32)
        nc.scalar.mul(msq[:], sqp[:], -1.0)
        pack2 = setup.tile([P, 2 * CPP], f16)                    # [msh | msl]
        nc.vector.tensor_copy(pack2[:, 0:CPP], msq[:])           # msh = fp16(-sq)
        mshf = setup.tile([P, CPP], f32)
        nc.vector.tensor_copy(mshf[:], pack2[:, 0:CPP])
        mslf = setup.tile([P, CPP], f32)
        nc.vector.tensor_sub(mslf[:], msq[:], mshf[:])
        nc.vector.tensor_copy(pack2[:, CPP:2 * CPP], mslf[:])    # msl

        # ---- PE transposes to candidate-row layout --------------------------
        # input free dims pre-permuted "(c d) -> (d c)" so transposed rows come
        # out coord-blocked: rows [32d + c] = coord d of point (c,p)
        def tr(src_view, pdim, dtype, ident):
            ps_t = pstr.tile([pdim, P], src_view.dtype, tag="tr")
            nc.tensor.transpose(ps_t[:], src_view, ident)
            sb = setup.tile([pdim, P], dtype)
            nc.scalar.copy(sb[:], ps_t[:])
            return sb

        # materialize the (d c) permutation first: the PE transpose input AP
        # must have a single free dimension on hardware
        hperm = setup.tile([P, 3 * CPP], f16)
        nc.vector.tensor_copy(hperm[:], h16[:].rearrange("p (c d) -> p d c", d=3))
        lperm = setup.tile([P, 3 * CPP], f16)
        nc.vector.tensor_copy(lperm[:], l16[:].rearrange("p (c d) -> p d c", d=3))
        fperm = setup.tile([P, 3 * CPP], f32)
        nc.gpsimd.tensor_copy(fperm[:], stage[:].rearrange("p (c d) -> p d c", d=3))

        Th = tr(hperm[:], 3 * CPP, f16, ident16[:])
        Tl = tr(lperm[:], 3 * CPP, f16, ident16[:])
        Tf = tr(fperm[:], 3 * CPP, f32, ident32[:])
        T2 = tr(pack2[:], 2 * CPP, f16, ident16[:])
        Tq = tr(sqp[:], CPP, f32, ident32[:])

        # ---- operand tiles [13, N] fp16 (rows via fast 256B-run DMAs) -------
        # matmul terms (contraction k):
        #   k 0-2 : 2h_i * h_j      k 3-5 : 2l_i * h_j     k 6-8 : 2h_i * l_j
        #   k 9-10: (-sq_i hi/lo)*1 k 11-12: 1*(-sq_j hi/lo)
        xrh = const.tile([13, N], f16)       # rhs  rows [h,h,l,1,msq]
        xlh = const.tile([13, N], f16)       # lhsT rows [2h,2l,2h,msq,1]
        # 2x-scaled transposed tiles (compute stays at partition offset 0;
        # odd-partition row placement goes through DMAs, which allow any
        # partition offset)
        Th2 = setup.tile([3 * CPP, P], f16)
        nc.vector.tensor_scalar_mul(Th2[:], Th[:], 2.0)          # exact in fp16
        Tl2 = setup.tile([3 * CPP, P], f16)
        nc.vector.tensor_scalar_mul(Tl2[:], Tl[:], 2.0)
        ones_row = const.tile([1, N], f16)
        nc.vector.memset(ones_row[:], 1.0)

        engs = (nc.sync, nc.scalar, nc.gpsimd)

        def row(dst_tile, r0, src):
            return dst_tile[r0:r0 + 1, :].rearrange("r (c p) -> r c p", c=CPP), src

        for d in range(3):
            engs[d].dma_start(*row(xrh, d, Th[32 * d:32 * (d + 1), :]))
            engs[d].dma_start(*row(xrh, 3 + d, Th[32 * d:32 * (d + 1), :]))
            engs[d].dma_start(*row(xrh, 6 + d, Tl[32 * d:32 * (d + 1), :]))
            engs[d].dma_start(*row(xlh, d, Th2[32 * d:32 * (d + 1), :]))
            engs[d].dma_start(*row(xlh, 3 + d, Tl2[32 * d:32 * (d + 1), :]))
            engs[d].dma_start(*row(xlh, 6 + d, Th2[32 * d:32 * (d + 1), :]))
        nc.sync.dma_start(xrh[9:10, :], ones_row[:])
        nc.scalar.dma_start(xrh[10:11, :], ones_row[:])
        nc.sync.dma_start(*row(xrh, 11, T2[0:CPP, :]))
        nc.scalar.dma_start(*row(xrh, 12, T2[CPP:2 * CPP, :]))
        nc.gpsimd.dma_start(*row(xlh, 9, T2[0:CPP, :]))
        nc.sync.dma_start(*row(xlh, 10, T2[CPP:2 * CPP, :]))
        nc.scalar.dma_start(xlh[11:12, :], ones_row[:])
        nc.gpsimd.dma_start(xlh[12:13, :], ones_row[:])

        # ---- gather table: coords on 16g+{0,1,2}, |x|^2 on 16g+3 ------------
        tbl = const.tile([P, N], f32)
        nc.gpsimd.memset(tbl[:], 0.0)
        for g in range(NG):
            eng = engs[g % 3]
            for d in range(3):
                eng.dma_start(
                    tbl[G16 * g + d:G16 * g + d + 1, :].rearrange("r (c p) -> r c p", c=CPP),
                    Tf[32 * d:32 * (d + 1), :])
            eng.dma_start(
                tbl[G16 * g + 3:G16 * g + 4, :].rearrange("r (c p) -> r c p", c=CPP),
                Tq[:])

        # ---- selection matmul rhs: E_sq (row 16g+3 -> +1), E_xyz (-1/5) -----
        esel_sq = const.tile([P, NG], f32)
        nc.vector.memset(esel_sq[:], 0.0)
        esel_xyz = const.tile([P, NG], f32)
        nc.vector.memset(esel_xyz[:], 0.0)
        one1 = const.tile([1, 1], f32)
        nc.vector.memset(one1[:], 1.0)
        mfifth = const.tile([3, 1], f32)
        nc.vector.memset(mfifth[:], -1.0 / KNN)
        for g in range(NG):
            engs[g % 3].dma_start(esel_sq[G16 * g + 3:G16 * g + 4, g:g + 1], one1[:])
            engs[g % 3].dma_start(esel_xyz[G16 * g:G16 * g + 3, g:g + 1], mfifth[:])

        trace_ps = psacc.tile([G16, NG * NBLK], f32)   # [16, 256], one bank

        # ---- main loop over row blocks -------------------------------------
        for r in range(NBLK):
            lhsT = xlh[:, r * P:(r + 1) * P]
            mval = mpool.tile([P, N], f32)
            for c in range(NCH):
                sl = slice(c * CH, (c + 1) * CH)
                ps = psum.tile([P, CH], f32, tag="mm")
                nc.tensor.matmul(ps[:], lhsT, xrh[:, sl], start=True, stop=True)
                nc.scalar.copy(mval[:, sl], ps[:])

            v8 = small.tile([P, 8], f32, tag="v8")
            nc.vector.max(v8[:], mval[:])
            idx8 = small.tile([P, 8], dt.uint16, tag="idx8")
            nc.vector.max_index(idx8[:], v8[:], mval[:])

            # gather: group g gathers, for its 16 queries, slot-major:
            # gath[p, s*16+q16] = tbl[p, idx8[16*(p//16)+q16, s]]
            gath = small.tile([P, KNN * G16], f32, tag="gath")
            nc.gpsimd.indirect_copy(gath[:], tbl[:], idx8[:, :KNN], True)

            t1 = small.tile([P, G16], f32, tag="t1")
            nc.gpsimd.tensor_add(t1[:], gath[:, 0:G16], gath[:, G16:2 * G16])
            t2 = small.tile([P, G16], f32, tag="t2")
            nc.gpsimd.tensor_add(t2[:], gath[:, 2 * G16:3 * G16],
                                 gath[:, 3 * G16:4 * G16])
            t3 = small.tile([P, G16], f32, tag="t3")
            nc.gpsimd.tensor_add(t3[:], t1[:], t2[:])
            S = small.tile([P, G16], f32, tag="S")
            nc.gpsimd.tensor_add(S[:], t3[:], gath[:, 4 * G16:5 * G16])
            S2 = small.tile([P, G16], f32, tag="S2")
            nc.gpsimd.tensor_mul(S2[:], S[:], S[:])

            osl = slice(r * NG, (r + 1) * NG)
            nc.tensor.matmul(trace_ps[:, osl], S[:], esel_sq[:],
                             start=True, stop=False)
            nc.tensor.matmul(trace_ps[:, osl], S2[:], esel_xyz[:],
                             start=False, stop=True)

        # ---- normalize + store ---------------------------------------------
        tr_sb = const.tile([G16, NG * NBLK], f32)
        nc.scalar.copy(tr_sb[:], trace_ps[:])
        gmax = const.tile([G16, 1], f32)
        nc.vector.tensor_reduce(gmax[:], tr_sb[:], axis=Axis.X, op=Alu.max)
        gmax_t_ps = pstr.tile([1, G16], f32, tag="tr")
        nc.tensor.transpose(gmax_t_ps[:], gmax[:], ident32[:G16, :G16])
        gmax_t = const.tile([1, G16], f32)
        nc.scalar.copy(gmax_t[:], gmax_t_ps[:])
        gmax1 = const.tile([1, 1], f32)
        nc.vector.tensor_reduce(gmax1[:], gmax_t[:], axis=Axis.X, op=Alu.max)
        denom1 = const.tile([1, 1], f32)
        nc.vector.tensor_scalar_add(denom1[:], gmax1[:], 1e-8)
        rec1 = const.tile([1, 1], f32)
        nc.vector.reciprocal(rec1[:], denom1[:])
        ones_col = const.tile([1, G16], f32)
        nc.vector.memset(ones_col[:], 1.0)
        rec_ps = pstr.tile([G16, 1], f32, tag="tr")
        nc.tensor.matmul(rec_ps[:], ones_col[:], rec1[:], start=True, stop=True)
        rec = const.tile([G16, 1], f32)
        nc.scalar.copy(rec[:], rec_ps[:])
        outv = const.tile([G16, NG * NBLK], f32)
        nc.vector.tensor_scalar_mul(outv[:], tr_sb[:], rec[:])

        # query n~ = 128r + 16g + q maps to original n = 512g + 32q + r
        nc.sync.dma_start(
            out_d.ap().rearrange("(g q r) -> q r g", g=NG, q=G16, r=NBLK),
            outv[:].rearrange("q (r g) -> q r g", r=NBLK, g=NG),
        )

    nc.compile()
    return nc


_NC_CACHE = {}


def kernel(pcd, k):
    pcd = np.asarray(pcd)
    k = int(np.asarray(k))
    assert k == KNN, f"kernel hardcodes k={KNN}, got {k}"
    B, n, d = pcd.shape
    assert (n, d) == (N, 3), f"kernel hardcodes N={N}, got {(n, d)}"

    from concourse.bass_utils import run_bass_kernel_spmd

    if "nc" not in _NC_CACHE:
        _NC_CACHE["nc"] = build_nc()
    nc = _NC_CACHE["nc"]

    in_maps = [{"pcd": np.ascontiguousarray(pcd[b], dtype=np.float32)}
               for b in range(B)]
    res = run_bass_kernel_spmd(nc, in_maps, list(range(B)))
    out = np.stack([res.results[b]["out"] for b in range(B)], axis=0)
    return out.astype(np.float32, copy=False)


if __name__ == "__main__":
    x = np.random.randn(8, N, 3).astype(np.float32)
    y = kernel(x, 5)
    print(y.shape, y.dtype, y[:2, :4])


# revision 17
# speedup vs baseline: 2.0370x; 1.0047x over previous
"""Trainium2 Bass kernel for batched 3-D k-NN local-covariance trace.

Problem: pcd [B=8, N=4096, 3] -> per-point trace of the 3x3 covariance of its
k=5 nearest neighbors (self included), normalized by the per-batch max.

Sharding: data-parallel over batch — core b owns batch b (N=4096 points).

Per-core algorithm (all SBUF-resident after the initial load):
  * rank value r[i,j] = 2*x_i.x_j - |x_i|^2 - |x_j|^2 = -d2[i,j], computed as
    an fp16 hi/lo-split augmented matmul (K=13) that streams 1 col/cycle on
    the PE (4x faster than fp32) while keeping ~fp32 accuracy: x = h + l with
    h = fp16(x), l = fp16(x - h); 2x_i*x_j = 2h_i*h_j + 2l_i*h_j + 2h_i*l_j
    (the dropped 2*l_i*l_j term is ~1e-6); |x|^2 split the same way.
  * top-5 neighbors per query via DVE max (top-8 values) + max_index.
  * neighbor gather via gpsimd indirect_copy from a table holding coords on
    partitions 16g+{0,1,2} and |x|^2 on 16g+3 — the sq row rides the same
    gather for free.
  * trace via S1/S2: trace = S_sq - |S1|^2/5, assembled by two tiny
    PSUM-accumulated selection matmuls per row block.
  * global max (gpsimd partition_all_reduce) -> scale -> DMA out.

Point order: the kernel works in transposed order n~ = (n%32)*128 + n//32
(n = original point index) because the setup pipeline stages pcd as
[128 partitions, 32 points x 3] and PE-transposes it; the selection/gather
are order-agnostic and the output DMA pattern maps back to original order.
"""

import numpy as np
from contextlib import ExitStack

N = 4096
KNN = 5
P = 128          # queries per row block
NBLK = N // P    # 32 row blocks
CH = 512         # candidate chunk (one fp32 PSUM bank)
NCH = N // CH    # 8 chunks
G16 = 16         # partitions per gpsimd core group
NG = P // G16    # 8 groups per row block
CPP = 32         # points staged per partition (N / 128)


def build_nc():
    import concourse.bass as bass
    import concourse.tile as tile
    from concourse import bacc, mybir
    from concourse import bass_isa

    dt = mybir.dt
    f32 = dt.float32
    f16 = dt.float16
    Alu = mybir.AluOpType
    Axis = mybir.AxisListType

    nc = bacc.Bacc("TRN2", target_bir_lowering=False, debug=False)
    pcd_d = nc.dram_tensor("pcd", [N, 3], f32, kind="ExternalInput")
    out_d = nc.dram_tensor("out", [N], f32, kind="ExternalOutput")

    with tile.TileContext(nc) as tc, ExitStack() as ctx:
        const = ctx.enter_context(tc.tile_pool(name="const", bufs=1))
        setup = ctx.enter_context(tc.tile_pool(name="setup", bufs=1))
        mpool = ctx.enter_context(tc.tile_pool(name="mval", bufs=3))
        small = ctx.enter_context(tc.tile_pool(name="small", bufs=3))
        psum = ctx.enter_context(tc.tile_pool(name="psum", bufs=6, space="PSUM"))
        pstr = ctx.enter_context(tc.tile_pool(name="pstr", bufs=1, space="PSUM"))
        psacc = ctx.enter_context(tc.tile_pool(name="psacc", bufs=1, space="PSUM"))

        # ---- stage pcd: one contiguous DMA, [p, c*3+d] = pcd[32p+c, d] ------
        stage = setup.tile([P, 3 * CPP], f32)
        nc.sync.dma_start(
            stage[:], pcd_d.ap().rearrange("(p c) d -> p (c d)", p=P, c=CPP))

        # identities for PE transposes (diagonal via affine_select)
        ident16 = const.tile([P, P], f16)
        nc.vector.memset(ident16[:], 1.0)
        nc.gpsimd.affine_select(ident16[:], ident16[:], [[1, P]],
                                Alu.is_equal, 0.0, base=0, channel_multiplier=-1)
        ident32 = const.tile([P, P], f32)
        nc.vector.memset(ident32[:], 1.0)
        nc.gpsimd.affine_select(ident32[:], ident32[:], [[1, P]],
                                Alu.is_equal, 0.0, base=0, channel_multiplier=-1)

        # ---- fp16 hi/lo split of coords (tiny [128, 96] elementwise ops) ----
        h16 = setup.tile([P, 3 * CPP], f16)
        nc.vector.tensor_copy(h16[:], stage[:])                  # h = fp16(x)
        hf = setup.tile([P, 3 * CPP], f32)
        nc.vector.tensor_copy(hf[:], h16[:])
        lf = setup.tile([P, 3 * CPP], f32)
        nc.vector.tensor_sub(lf[:], stage[:], hf[:])             # l = x - h
        l16 = setup.tile([P, 3 * CPP], f16)
        nc.vector.tensor_copy(l16[:], lf[:])

        # ---- |x|^2 per point, then hi/lo split of -sq -----------------------
        sqc = setup.tile([P, 3 * CPP], f32)
        nc.scalar.square(sqc[:], stage[:])
        sqp = setup.tile([P, CPP], f32)                          # +|x|^2
        nc.vector.tensor_reduce(sqp[:], sqc[:].rearrange("p (c d) -> p c d", d=3),
                                axis=Axis.X, op=Alu.add)
        msq = setup.tile([P, CPP], f32)
        nc.scalar.mul(msq[:], sqp[:], -1.0)
        pack2 = setup.tile([P, 2 * CPP], f16)                    # [msh | msl]
        nc.vector.tensor_copy(pack2[:, 0:CPP], msq[:])           # msh = fp16(-sq)
        mshf = setup.tile([P, CPP], f32)
        nc.vector.tensor_copy(mshf[:], pack2[:, 0:CPP])
        mslf = setup.tile([P, CPP], f32)
        nc.vector.tensor_sub(mslf[:], msq[:], mshf[:])
        nc.vector.tensor_copy(pack2[:, CPP:2 * CPP], mslf[:])    # msl

        # ---- PE transposes to candidate-row layout --------------------------
        # input free dims pre-permuted "(c d) -> (d c)" so transposed rows come
        # out coord-blocked: rows [32d + c] = coord d of point (c,p)
        def tr(src_view, pdim, dtype, ident):
            ps_t = pstr.tile([pdim, P], src_view.dtype, tag="tr")
            nc.tensor.transpose(ps_t[:], src_view, ident)
            sb = setup.tile([pdim, P], dtype)
            nc.scalar.copy(sb[:], ps_t[:])
            return sb

        # materialize the (d c) permutation first: the PE transpose input AP
        # must have a single free dimension on hardware
        hperm = setup.tile([P, 3 * CPP], f16)
        nc.vector.tensor_copy(hperm[:], h16[:].rearrange("p (c d) -> p d c", d=3))
        lperm = setup.tile([P, 3 * CPP], f16)
        nc.vector.tensor_copy(lperm[:], l16[:].rearrange("p (c d) -> p d c", d=3))
        fperm = setup.tile([P, 3 * CPP], f32)
        nc.gpsimd.tensor_copy(fperm[:], stage[:].rearrange("p (c d) -> p d c", d=3))

        Th = tr(hperm[:], 3 * CPP, f16, ident16[:])
        Tl = tr(lperm[:], 3 * CPP, f16, ident16[:])
        Tf = tr(fperm[:], 3 * CPP, f32, ident32[:])
        T2 = tr(pack2[:], 2 * CPP, f16, ident16[:])
        Tq = tr(sqp[:], CPP, f32, ident32[:])

        # ---- operand tiles [13, N] fp16 (rows via fast 256B-run DMAs) -------
        # matmul terms (contraction k):
        #   k 0-2 : 2h_i * h_j      k 3-5 : 2l_i * h_j     k 6-8 : 2h_i * l_j
        #   k 9-10: (-sq_i hi/lo)*1 k 11-12: 1*(-sq_j hi/lo)
        xrh = const.tile([13, N], f16)       # rhs  rows [h,h,l,1,msq]
        xlh = const.tile([13, N], f16)       # lhsT rows [2h,2l,2h,msq,1]
        # 2x-scaled transposed tiles (compute stays at partition offset 0;
        # odd-partition row placement goes through DMAs, which allow any
        # partition offset)
        Th2 = setup.tile([3 * CPP, P], f16)
        nc.vector.tensor_scalar_mul(Th2[:], Th[:], 2.0)          # exact in fp16
        Tl2 = setup.tile([3 * CPP, P], f16)
        nc.vector.tensor_scalar_mul(Tl2[:], Tl[:], 2.0)
        ones_row = const.tile([1, N], f16)
        nc.vector.memset(ones_row[:], 1.0)

        engs = (nc.sync, nc.gpsimd)

        def row(dst_tile, r0, src):
            return dst_tile[r0:r0 + 1, :].rearrange("r (c p) -> r c p", c=CPP), src

        for d in range(3):
            engs[d % 2].dma_start(*row(xrh, d, Th[32 * d:32 * (d + 1), :]))
            engs[(d + 1) % 2].dma_start(*row(xrh, 3 + d, Th[32 * d:32 * (d + 1), :]))
            engs[d % 2].dma_start(*row(xrh, 6 + d, Tl[32 * d:32 * (d + 1), :]))
            engs[(d + 1) % 2].dma_start(*row(xlh, d, Th2[32 * d:32 * (d + 1), :]))
            engs[d % 2].dma_start(*row(xlh, 3 + d, Tl2[32 * d:32 * (d + 1), :]))
            engs[(d + 1) % 2].dma_start(*row(xlh, 6 + d, Th2[32 * d:32 * (d + 1), :]))
        nc.sync.dma_start(xrh[9:10, :], ones_row[:])
        nc.gpsimd.dma_start(xrh[10:11, :], ones_row[:])
        nc.sync.dma_start(*row(xrh, 11, T2[0:CPP, :]))
        nc.gpsimd.dma_start(*row(xrh, 12, T2[CPP:2 * CPP, :]))
        nc.gpsimd.dma_start(*row(xlh, 9, T2[0:CPP, :]))
        nc.sync.dma_start(*row(xlh, 10, T2[CPP:2 * CPP, :]))
        nc.sync.dma_start(xlh[11:12, :], ones_row[:])
        nc.gpsimd.dma_start(xlh[12:13, :], ones_row[:])

        # ---- gather table: coords on 16g+{0,1,2}, |x|^2 on 16g+3 ------------
        tbl = const.tile([P, N], f32)
        nc.gpsimd.memset(tbl[:], 0.0)
        for g in range(NG):
            eng = engs[g % 2]
            for d in range(3):
                eng.dma_start(
                    tbl[G16 * g + d:G16 * g + d + 1, :].rearrange("r (c p) -> r c p", c=CPP),
                    Tf[32 * d:32 * (d + 1), :])
            eng.dma_start(
                tbl[G16 * g + 3:G16 * g + 4, :].rearrange("r (c p) -> r c p", c=CPP),
                Tq[:])

        # ---- selection matmul rhs: E_sq (row 16g+3 -> +1), E_xyz (-1/5) -----
        esel_sq = const.tile([P, NG], f32)
        nc.vector.memset(esel_sq[:], 0.0)
        esel_xyz = const.tile([P, NG], f32)
        nc.vector.memset(esel_xyz[:], 0.0)
        one1 = const.tile([1, 1], f32)
        nc.vector.memset(one1[:], 1.0)
        mfifth = const.tile([3, 1], f32)
        nc.vector.memset(mfifth[:], -1.0 / KNN)
        for g in range(NG):
            engs[g % 2].dma_start(esel_sq[G16 * g + 3:G16 * g + 4, g:g + 1], one1[:])
            engs[(g + 1) % 2].dma_start(esel_xyz[G16 * g:G16 * g + 3, g:g + 1], mfifth[:])

        trace_ps = psacc.tile([G16, NG * NBLK], f32)   # [16, 256], one bank

        # ---- main loop over row blocks -------------------------------------
        for r in range(NBLK):
            lhsT = xlh[:, r * P:(r + 1) * P]
            mval = mpool.tile([P, N], f32)
            for c in range(NCH):
                sl = slice(c * CH, (c + 1) * CH)
                ps = psum.tile([P, CH], f32, tag="mm")
                nc.tensor.matmul(ps[:], lhsT, xrh[:, sl], start=True, stop=True)
                nc.scalar.copy(mval[:, sl], ps[:])

            v8 = small.tile([P, 8], f32, tag="v8")
            nc.vector.max(v8[:], mval[:])
            idx8 = small.tile([P, 8], dt.uint16, tag="idx8")
            nc.vector.max_index(idx8[:], v8[:], mval[:])

            # gather: group g gathers, for its 16 queries, slot-major:
            # gath[p, s*16+q16] = tbl[p, idx8[16*(p//16)+q16, s]]
            gath = small.tile([P, KNN * G16], f32, tag="gath")
            nc.gpsimd.indirect_copy(gath[:], tbl[:], idx8[:, :KNN], True)

            t1 = small.tile([P, G16], f32, tag="t1")
            nc.gpsimd.tensor_add(t1[:], gath[:, 0:G16], gath[:, G16:2 * G16])
            t2 = small.tile([P, G16], f32, tag="t2")
            nc.gpsimd.tensor_add(t2[:], gath[:, 2 * G16:3 * G16],
                                 gath[:, 3 * G16:4 * G16])
            t3 = small.tile([P, G16], f32, tag="t3")
            nc.gpsimd.tensor_add(t3[:], t1[:], t2[:])
            S = small.tile([P, G16], f32, tag="S")
            nc.gpsimd.tensor_add(S[:], t3[:], gath[:, 4 * G16:5 * G16])
            S2 = small.tile([P, G16], f32, tag="S2")
            nc.gpsimd.tensor_mul(S2[:], S[:], S[:])

            osl = slice(r * NG, (r + 1) * NG)
            nc.tensor.matmul(trace_ps[:, osl], S[:], esel_sq[:],
                             start=True, stop=False)
            nc.tensor.matmul(trace_ps[:, osl], S2[:], esel_xyz[:],
                             start=False, stop=True)

        # ---- normalize + store ---------------------------------------------
        tr_sb = const.tile([G16, NG * NBLK], f32)
        nc.scalar.copy(tr_sb[:], trace_ps[:])
        gmax = const.tile([G16, 1], f32)
        nc.vector.tensor_reduce(gmax[:], tr_sb[:], axis=Axis.X, op=Alu.max)
        gmax_t_ps = pstr.tile([1, G16], f32, tag="tr")
        nc.tensor.transpose(gmax_t_ps[:], gmax[:], ident32[:G16, :G16])
        gmax_t = const.tile([1, G16], f32)
        nc.scalar.copy(gmax_t[:], gmax_t_ps[:])
        gmax1 = const.tile([1, 1], f32)
        nc.vector.tensor_reduce(gmax1[:], gmax_t[:], axis=Axis.X, op=Alu.max)
        denom1 = const.tile([1, 1], f32)
        nc.vector.tensor_scalar_add(denom1[:], gmax1[:], 1e-8)
        rec1 = const.tile([1, 1], f32)
        nc.vector.reciprocal(rec1[:], denom1[:])
        ones_col = const.tile([1, G16], f32)
        nc.vector.memset(ones_col[:], 1.0)
        rec_ps = pstr.tile([G16, 1], f32, tag="tr")
        nc.tensor.matmul(rec_ps[:], ones_col[:], rec1[:], start=True, stop=True)
        rec = const.tile([G16, 1], f32)
        nc.scalar.copy(rec[:], rec_ps[:])
        outv = const.tile([G16, NG * NBLK], f32)
        nc.vector.tensor_scalar_mul(outv[:], tr_sb[:], rec[:])

        # query n~ = 128r + 16g + q maps to original n = 512g + 32q + r
        nc.sync.dma_start(
            out_d.ap().rearrange("(g q r) -> q r g", g=NG, q=G16, r=NBLK),
            outv[:].rearrange("q (r g) -> q r g", r=NBLK, g=NG),
        )

    nc.compile()
    return nc


_NC_CACHE = {}


def kernel(pcd, k):
    pcd = np.asarray(pcd)
    k = int(np.asarray(k))
    assert k == KNN, f"kernel hardcodes k={KNN}, got {k}"
    B, n, d = pcd.shape
    assert (n, d) == (N, 3), f"kernel hardcodes N={N}, got {(n, d)}"

    from concourse.bass_utils import run_bass_kernel_spmd

    if "nc" not in _NC_CACHE:
        _NC_CACHE["nc"] = build_nc()
    nc = _NC_CACHE["nc"]

    in_maps = [{"pcd": np.ascontiguousarray(pcd[b], dtype=np.float32)}
               for b in range(B)]
    res = run_bass_kernel_spmd(nc, in_maps, list(range(B)))
    out = np.stack([res.results[b]["out"] for b in range(B)], axis=0)
    return out.astype(np.float32, copy=False)


if __name__ == "__main__":
    x = np.random.randn(8, N, 3).astype(np.float32)
    y = kernel(x, 5)
    print(y.shape, y.dtype, y[:2, :4])


# revision 18
# speedup vs baseline: 2.0416x; 1.0023x over previous
"""Trainium2 Bass kernel for batched 3-D k-NN local-covariance trace.

Problem: pcd [B=8, N=4096, 3] -> per-point trace of the 3x3 covariance of its
k=5 nearest neighbors (self included), normalized by the per-batch max.

Sharding: data-parallel over batch — core b owns batch b (N=4096 points).

Per-core algorithm (all SBUF-resident after the initial load):
  * rank value r[i,j] = 2*x_i.x_j - |x_i|^2 - |x_j|^2 = -d2[i,j], computed as
    an fp16 hi/lo-split augmented matmul (K=13) that streams 1 col/cycle on
    the PE (4x faster than fp32) while keeping ~fp32 accuracy: x = h + l with
    h = fp16(x), l = fp16(x - h); 2x_i*x_j = 2h_i*h_j + 2l_i*h_j + 2h_i*l_j
    (the dropped 2*l_i*l_j term is ~1e-6); |x|^2 split the same way.
  * top-5 neighbors per query via DVE max (top-8 values) + max_index.
  * neighbor gather via gpsimd indirect_copy from a table holding coords on
    partitions 16g+{0,1,2} and |x|^2 on 16g+3 — the sq row rides the same
    gather for free.
  * trace via S1/S2: trace = S_sq - |S1|^2/5, assembled by two tiny
    PSUM-accumulated selection matmuls per row block.
  * global max (gpsimd partition_all_reduce) -> scale -> DMA out.

Point order: the kernel works in transposed order n~ = (n%32)*128 + n//32
(n = original point index) because the setup pipeline stages pcd as
[128 partitions, 32 points x 3] and PE-transposes it; the selection/gather
are order-agnostic and the output DMA pattern maps back to original order.
"""

import numpy as np
from contextlib import ExitStack

N = 4096
KNN = 5
P = 128          # queries per row block
NBLK = N // P    # 32 row blocks
CH = 512         # candidate chunk (one fp32 PSUM bank)
NCH = N // CH    # 8 chunks
G16 = 16         # partitions per gpsimd core group
NG = P // G16    # 8 groups per row block
CPP = 32         # points staged per partition (N / 128)


def build_nc():
    import concourse.bass as bass
    import concourse.tile as tile
    from concourse import bacc, mybir
    from concourse import bass_isa

    dt = mybir.dt
    f32 = dt.float32
    f16 = dt.float16
    Alu = mybir.AluOpType
    Axis = mybir.AxisListType

    nc = bacc.Bacc("TRN2", target_bir_lowering=False, debug=False)
    pcd_d = nc.dram_tensor("pcd", [N, 3], f32, kind="ExternalInput")
    out_d = nc.dram_tensor("out", [N], f32, kind="ExternalOutput")

    with tile.TileContext(nc) as tc, ExitStack() as ctx:
        const = ctx.enter_context(tc.tile_pool(name="const", bufs=1))
        setup = ctx.enter_context(tc.tile_pool(name="setup", bufs=1))
        mpool = ctx.enter_context(tc.tile_pool(name="mval", bufs=3))
        small = ctx.enter_context(tc.tile_pool(name="small", bufs=3))
        psum = ctx.enter_context(tc.tile_pool(name="psum", bufs=5, space="PSUM"))
        pstr = ctx.enter_context(tc.tile_pool(name="pstr", bufs=2, space="PSUM"))
        psacc = ctx.enter_context(tc.tile_pool(name="psacc", bufs=1, space="PSUM"))

        # ---- stage pcd: one contiguous DMA, [p, c*3+d] = pcd[32p+c, d] ------
        stage = setup.tile([P, 3 * CPP], f32)
        nc.sync.dma_start(
            stage[:], pcd_d.ap().rearrange("(p c) d -> p (c d)", p=P, c=CPP))

        # identities for PE transposes (diagonal via affine_select)
        ident16 = const.tile([P, P], f16)
        nc.vector.memset(ident16[:], 1.0)
        nc.gpsimd.affine_select(ident16[:], ident16[:], [[1, P]],
                                Alu.is_equal, 0.0, base=0, channel_multiplier=-1)
        ident32 = const.tile([P, P], f32)
        nc.vector.memset(ident32[:], 1.0)
        nc.gpsimd.affine_select(ident32[:], ident32[:], [[1, P]],
                                Alu.is_equal, 0.0, base=0, channel_multiplier=-1)

        # ---- fp16 hi/lo split of coords (tiny [128, 96] elementwise ops) ----
        h16 = setup.tile([P, 3 * CPP], f16)
        nc.vector.tensor_copy(h16[:], stage[:])                  # h = fp16(x)
        hf = setup.tile([P, 3 * CPP], f32)
        nc.vector.tensor_copy(hf[:], h16[:])
        lf = setup.tile([P, 3 * CPP], f32)
        nc.vector.tensor_sub(lf[:], stage[:], hf[:])             # l = x - h
        l16 = setup.tile([P, 3 * CPP], f16)
        nc.vector.tensor_copy(l16[:], lf[:])

        # ---- |x|^2 per point, then hi/lo split of -sq -----------------------
        sqc = setup.tile([P, 3 * CPP], f32)
        nc.scalar.square(sqc[:], stage[:])
        sqp = setup.tile([P, CPP], f32)                          # +|x|^2
        nc.vector.tensor_reduce(sqp[:], sqc[:].rearrange("p (c d) -> p c d", d=3),
                                axis=Axis.X, op=Alu.add)
        msq = setup.tile([P, CPP], f32)
        nc.scalar.mul(msq[:], sqp[:], -1.0)
        pack2 = setup.tile([P, 2 * CPP], f16)                    # [msh | msl]
        nc.vector.tensor_copy(pack2[:, 0:CPP], msq[:])           # msh = fp16(-sq)
        mshf = setup.tile([P, CPP], f32)
        nc.vector.tensor_copy(mshf[:], pack2[:, 0:CPP])
        mslf = setup.tile([P, CPP], f32)
        nc.vector.tensor_sub(mslf[:], msq[:], mshf[:])
        nc.vector.tensor_copy(pack2[:, CPP:2 * CPP], mslf[:])    # msl

        # ---- PE transposes to candidate-row layout --------------------------
        # input free dims pre-permuted "(c d) -> (d c)" so transposed rows come
        # out coord-blocked: rows [32d + c] = coord d of point (c,p)
        def tr(src_view, pdim, dtype, ident):
            ps_t = pstr.tile([pdim, P], src_view.dtype, tag="tr")
            nc.tensor.transpose(ps_t[:], src_view, ident)
            sb = setup.tile([pdim, P], dtype)
            nc.scalar.copy(sb[:], ps_t[:])
            return sb

        # materialize the (d c) permutation first: the PE transpose input AP
        # must have a single free dimension on hardware
        hperm = setup.tile([P, 3 * CPP], f16)
        nc.vector.tensor_copy(hperm[:], h16[:].rearrange("p (c d) -> p d c", d=3))
        lperm = setup.tile([P, 3 * CPP], f16)
        nc.vector.tensor_copy(lperm[:], l16[:].rearrange("p (c d) -> p d c", d=3))
        fperm = setup.tile([P, 3 * CPP], f32)
        nc.gpsimd.tensor_copy(fperm[:], stage[:].rearrange("p (c d) -> p d c", d=3))

        Th = tr(hperm[:], 3 * CPP, f16, ident16[:])
        Tl = tr(lperm[:], 3 * CPP, f16, ident16[:])
        Tf = tr(fperm[:], 3 * CPP, f32, ident32[:])
        T2 = tr(pack2[:], 2 * CPP, f16, ident16[:])
        Tq = tr(sqp[:], CPP, f32, ident32[:])

        # ---- operand tiles [13, N] fp16 (rows via fast 256B-run DMAs) -------
        # matmul terms (contraction k):
        #   k 0-2 : 2h_i * h_j      k 3-5 : 2l_i * h_j     k 6-8 : 2h_i * l_j
        #   k 9-10: (-sq_i hi/lo)*1 k 11-12: 1*(-sq_j hi/lo)
        xrh = const.tile([13, N], f16)       # rhs  rows [h,h,l,1,msq]
        xlh = const.tile([13, N], f16)       # lhsT rows [2h,2l,2h,msq,1]
        # 2x-scaled transposed tiles (compute stays at partition offset 0;
        # odd-partition row placement goes through DMAs, which allow any
        # partition offset)
        Th2 = setup.tile([3 * CPP, P], f16)
        nc.vector.tensor_scalar_mul(Th2[:], Th[:], 2.0)          # exact in fp16
        Tl2 = setup.tile([3 * CPP, P], f16)
        nc.vector.tensor_scalar_mul(Tl2[:], Tl[:], 2.0)
        ones_row = const.tile([1, N], f16)
        nc.vector.memset(ones_row[:], 1.0)

        engs = (nc.sync, nc.gpsimd)

        def row(dst_tile, r0, src):
            return dst_tile[r0:r0 + 1, :].rearrange("r (c p) -> r c p", c=CPP), src

        for d in range(3):
            engs[d % 2].dma_start(*row(xrh, d, Th[32 * d:32 * (d + 1), :]))
            engs[(d + 1) % 2].dma_start(*row(xrh, 3 + d, Th[32 * d:32 * (d + 1), :]))
            engs[d % 2].dma_start(*row(xrh, 6 + d, Tl[32 * d:32 * (d + 1), :]))
            engs[(d + 1) % 2].dma_start(*row(xlh, d, Th2[32 * d:32 * (d + 1), :]))
            engs[d % 2].dma_start(*row(xlh, 3 + d, Tl2[32 * d:32 * (d + 1), :]))
            engs[(d + 1) % 2].dma_start(*row(xlh, 6 + d, Th2[32 * d:32 * (d + 1), :]))
        nc.sync.dma_start(xrh[9:10, :], ones_row[:])
        nc.gpsimd.dma_start(xrh[10:11, :], ones_row[:])
        nc.sync.dma_start(*row(xrh, 11, T2[0:CPP, :]))
        nc.gpsimd.dma_start(*row(xrh, 12, T2[CPP:2 * CPP, :]))
        nc.gpsimd.dma_start(*row(xlh, 9, T2[0:CPP, :]))
        nc.sync.dma_start(*row(xlh, 10, T2[CPP:2 * CPP, :]))
        nc.sync.dma_start(xlh[11:12, :], ones_row[:])
        nc.gpsimd.dma_start(xlh[12:13, :], ones_row[:])

        # ---- gather table: coords on 16g+{0,1,2}, |x|^2 on 16g+3 ------------
        tbl = const.tile([P, N], f32)
        nc.gpsimd.memset(tbl[:], 0.0)
        for g in range(NG):
            eng = engs[g % 2]
            for d in range(3):
                eng.dma_start(
                    tbl[G16 * g + d:G16 * g + d + 1, :].rearrange("r (c p) -> r c p", c=CPP),
                    Tf[32 * d:32 * (d + 1), :])
            eng.dma_start(
                tbl[G16 * g + 3:G16 * g + 4, :].rearrange("r (c p) -> r c p", c=CPP),
                Tq[:])

        # ---- selection matmul rhs: E_sq (row 16g+3 -> +1), E_xyz (-1/5) -----
        esel_sq = const.tile([P, NG], f32)
        nc.vector.memset(esel_sq[:], 0.0)
        esel_xyz = const.tile([P, NG], f32)
        nc.vector.memset(esel_xyz[:], 0.0)
        one1 = const.tile([1, 1], f32)
        nc.vector.memset(one1[:], 1.0)
        mfifth = const.tile([3, 1], f32)
        nc.vector.memset(mfifth[:], -1.0 / KNN)
        for g in range(NG):
            engs[g % 2].dma_start(esel_sq[G16 * g + 3:G16 * g + 4, g:g + 1], one1[:])
            engs[(g + 1) % 2].dma_start(esel_xyz[G16 * g:G16 * g + 3, g:g + 1], mfifth[:])

        trace_ps = psacc.tile([G16, NG * NBLK], f32)   # [16, 256], one bank

        # ---- main loop over row blocks -------------------------------------
        for r in range(NBLK):
            lhsT = xlh[:, r * P:(r + 1) * P]
            mval = mpool.tile([P, N], f32)
            for c in range(NCH):
                sl = slice(c * CH, (c + 1) * CH)
                ps = psum.tile([P, CH], f32, tag="mm")
                nc.tensor.matmul(ps[:], lhsT, xrh[:, sl], start=True, stop=True)
                nc.scalar.copy(mval[:, sl], ps[:])

            v8 = small.tile([P, 8], f32, tag="v8")
            if r == 0:
                # warmup: scan halves as their copies land, merge candidates
                vA = small.tile([P, 8], f32, tag="vA")
                nc.vector.max(vA[:], mval[:, 0:N // 2])
                vB = small.tile([P, 16], f32, tag="vB")
                nc.vector.max(vB[:, 8:16], mval[:, N // 2:N])
                nc.vector.tensor_copy(vB[:, 0:8], vA[:])
                nc.vector.max(v8[:], vB[:])
            else:
                nc.vector.max(v8[:], mval[:])
            idx8 = small.tile([P, 8], dt.uint16, tag="idx8")
            nc.vector.max_index(idx8[:], v8[:], mval[:])

            # gather: group g gathers, for its 16 queries, slot-major:
            # gath[p, s*16+q16] = tbl[p, idx8[16*(p//16)+q16, s]]
            gath = small.tile([P, KNN * G16], f32, tag="gath")
            nc.gpsimd.indirect_copy(gath[:], tbl[:], idx8[:, :KNN], True)

            t1 = small.tile([P, G16], f32, tag="t1")
            nc.gpsimd.tensor_add(t1[:], gath[:, 0:G16], gath[:, G16:2 * G16])
            t2 = small.tile([P, G16], f32, tag="t2")
            nc.gpsimd.tensor_add(t2[:], gath[:, 2 * G16:3 * G16],
                                 gath[:, 3 * G16:4 * G16])
            t3 = small.tile([P, G16], f32, tag="t3")
            nc.gpsimd.tensor_add(t3[:], t1[:], t2[:])
            S = small.tile([P, G16], f32, tag="S")
            nc.gpsimd.tensor_add(S[:], t3[:], gath[:, 4 * G16:5 * G16])
            S2 = small.tile([P, G16], f32, tag="S2")
            nc.gpsimd.tensor_mul(S2[:], S[:], S[:])

            osl = slice(r * NG, (r + 1) * NG)
            nc.tensor.matmul(trace_ps[:, osl], S[:], esel_sq[:],
                             start=True, stop=False)
            nc.tensor.matmul(trace_ps[:, osl], S2[:], esel_xyz[:],
                             start=False, stop=True)

        # ---- normalize + store ---------------------------------------------
        tr_sb = const.tile([G16, NG * NBLK], f32)
        nc.scalar.copy(tr_sb[:], trace_ps[:])
        gmax = const.tile([G16, 1], f32)
        nc.vector.tensor_reduce(gmax[:], tr_sb[:], axis=Axis.X, op=Alu.max)
        gmax_t_ps = pstr.tile([1, G16], f32, tag="tr")
        nc.tensor.transpose(gmax_t_ps[:], gmax[:], ident32[:G16, :G16])
        gmax_t = const.tile([1, G16], f32)
        nc.scalar.copy(gmax_t[:], gmax_t_ps[:])
        gmax1 = const.tile([1, 1], f32)
        nc.vector.tensor_reduce(gmax1[:], gmax_t[:], axis=Axis.X, op=Alu.max)
        denom1 = const.tile([1, 1], f32)
        nc.vector.tensor_scalar_add(denom1[:], gmax1[:], 1e-8)
        rec1 = const.tile([1, 1], f32)
        nc.vector.reciprocal(rec1[:], denom1[:])
        ones_col = const.tile([1, G16], f32)
        nc.vector.memset(ones_col[:], 1.0)
        rec_ps = pstr.tile([G16, 1], f32, tag="tr")
        nc.tensor.matmul(rec_ps[:], ones_col[:], rec1[:], start=True, stop=True)
        rec = const.tile([G16, 1], f32)
        nc.scalar.copy(rec[:], rec_ps[:])
        outv = const.tile([G16, NG * NBLK], f32)
        nc.vector.tensor_scalar_mul(outv[:], tr_sb[:], rec[:])

        # query n~ = 128r + 16g + q maps to original n = 512g + 32q + r
        nc.sync.dma_start(
            out_d.ap().rearrange("(g q r) -> q r g", g=NG, q=G16, r=NBLK),
            outv[:].rearrange("q (r g) -> q r g", r=NBLK, g=NG),
        )

    nc.compile()
    return nc


_NC_CACHE = {}


def kernel(pcd, k):
    pcd = np.asarray(pcd)
    k = int(np.asarray(k))
    assert k == KNN, f"kernel hardcodes k={KNN}, got {k}"
    B, n, d = pcd.shape
    assert (n, d) == (N, 3), f"kernel hardcodes N={N}, got {(n, d)}"

    from concourse.bass_utils import run_bass_kernel_spmd

    if "nc" not in _NC_CACHE:
        _NC_CACHE["nc"] = build_nc()
    nc = _NC_CACHE["nc"]

    in_maps = [{"pcd": np.ascontiguousarray(pcd[b], dtype=np.float32)}
               for b in range(B)]
    res = run_bass_kernel_spmd(nc, in_maps, list(range(B)))
    out = np.stack([res.results[b]["out"] for b in range(B)], axis=0)
    return out.astype(np.float32, copy=False)


if __name__ == "__main__":
    x = np.random.randn(8, N, 3).astype(np.float32)
    y = kernel(x, 5)
    print(y.shape, y.dtype, y[:2, :4])
